# revision 2
# baseline (speedup 1.0000x reference)
"""CogVLM VisionExpert MLP (moe routing) on 8 trn2 NeuronCores.

Strategy:
  - Host computes the vision/language routing mask in numpy and permutes
    tokens by expert. Each token goes through exactly ONE expert (the
    reference computes both and selects; routing halves the matmul work).
  - Default sharding (expert-affinity DP4 x TP2): four 2-core tensor-parallel
    pairs, each pair owns one expert and a contiguous slice of that expert's
    tokens (for this problem's ~3:1 lang:vis split: 3 lang pairs + 1 vis
    pair). TP halves split the intermediate dim I=11008 -> 5504 = 43*128
    exactly, so the icol padding TP8 needs (1376->1408, +2.3% matmuls)
    vanishes, and every token block is >=~512 wide so the per-block weight
    stream hides under the matmuls. Host sums each pair's two partial
    outputs (the "all-reduce") and un-permutes.
  - Fallback (TP8 over I, all tokens on every core) when the expert split is
    too skewed for affinity pairs to balance.
  - bf16 matmuls with fp32 PSUM accumulation; no token padding anywhere (the
    matmul moving dim accepts any width <= 512; ragged blocks use balanced
    sub-widths so no tiny trailing matmuls).

Layouts shipped to the device (everything lands in SBUF with natural
[128-partition, free] shape and fully-contiguous DMA runs):
  xt   [NK, 128, T]       bf16   xt[k, p, t]       = X_perm[t, k*128+p]
  g/u  [ni, 128, NK*128]  bf16   g[it, p, k*128+c] = Wg_sh[k*128+p, it*128+c]
  d    [NK, 128, ni*128]  bf16   d[h, p, i*128+c]  = Wd_sh[i*128+p, h*128+c]
  yt   [NK, 128, T]       f32    yt[h, p, t]       = Y_part[t, h*128+p]
"""
import os
import numpy as np
import ml_dtypes

import concourse.bacc as bacc
import concourse.mybir as mybir
import concourse.tile as tile
from concourse.bass_utils import run_bass_kernel_spmd
from concourse.bass_interp import get_hw_module

bf16 = ml_dtypes.bfloat16
B, S, H, I = 2, 2048, 4096, 11008
NCORES = 8
NK = H // 128                # 32 h tiles
TSUB = 512                   # matmul moving-dim chunk (one PSUM bank of f32)
VISION_TOKEN_TYPE = 1

# TP8 fallback geometry
ISH8 = I // 8                # 1376 real icols per core
NI8 = (ISH8 + 127) // 128    # 11 tiles (padded to 1408)
# expert-affinity DP4 x TP2 geometry
ISH2 = I // 2                # 5504 icols per core
NI2 = ISH2 // 128            # 43 tiles, exact

FP32 = mybir.dt.float32
BF16 = mybir.dt.bfloat16

_nc_cache = {}

# observability for test harnesses (not used by grading)
last_results = None
last_run = None  # (nc, in_maps) of the most recent kernel() call


def _plan_blocks(n, tblk):
    """Split a token count into blocks of <= tblk+128 (weights re-stream once
    per block). A remainder <=128 is folded into the last block: a standalone
    narrow block starves the PE on its weight stream."""
    out = []
    t = 0
    while n - t >= tblk:
        out.append((t, tblk))
        t += tblk
    r = n - t
    if r > 0:
        if r <= 128 and out:
            t0, tc = out[-1]
            out[-1] = (t0, tc + r)
        else:
            out.append((t, r))
    return out


def _subs(tcols):
    """Split a block into balanced matmul moving-dim pieces (<= 512 each),
    e.g. 1057 -> 353/352/352 — avoids tiny trailing matmuls whose LDWEIGHTS
    cost can't hide under the streaming."""
    nsub = (tcols + TSUB - 1) // TSUB
    base, rem = divmod(tcols, nsub)
    out = []
    c = 0
    for s in range(nsub):
        w = base + (1 if s < rem else 0)
        out.append((c, w))
        c += w
    return out


def _build(Tt, ni, weight_sets, blocks, a_bufs, wd_bufs, y_bufs):
    """Emit the SPMD program: for each (set_key, t0, tcols) block run the
    swiglu MLP over that token window with that weight set's shards."""
    nc = bacc.Bacc("TRN2", target_bir_lowering=False, debug=False,
                   num_devices=NCORES)

    xt_d = nc.dram_tensor("xt", [NK, 128, Tt], BF16, kind="ExternalInput")
    w_d = {}
    for key, (gn, un, dn) in weight_sets.items():
        w_d[key] = (
            nc.dram_tensor(gn, [ni, 128, NK * 128], BF16, kind="ExternalInput"),
            nc.dram_tensor(un, [ni, 128, NK * 128], BF16, kind="ExternalInput"),
            nc.dram_tensor(dn, [NK, 128, ni * 128], BF16, kind="ExternalInput"),
        )
    yt_d = nc.dram_tensor("yt", [NK, 128, Tt], FP32, kind="ExternalOutput")

    with tile.TileContext(nc) as tc_:
        with (
            tc_.tile_pool(name="xp", bufs=1) as xp,
            tc_.tile_pool(name="apool", bufs=a_bufs) as apool,
            tc_.tile_pool(name="wgu", bufs=3) as wgu,
            tc_.tile_pool(name="wdp", bufs=wd_bufs) as wdp,
            tc_.tile_pool(name="sp", bufs=2) as sp,
            tc_.tile_pool(name="yp", bufs=y_bufs) as yp,
            tc_.tile_pool(name="pg", bufs=2, space="PSUM") as pgp,
            tc_.tile_pool(name="pu", bufs=2, space="PSUM") as pup,
            tc_.tile_pool(name="py", bufs=4, space="PSUM") as pyp,
        ):
            for bi, (key, t0, tcols) in enumerate(blocks):
                g_d, u_d, d_d = w_d[key]
                subs = _subs(tcols)
                # issue the first gate/up weight DMAs before the x block so
                # the opening matmuls aren't queued behind the x traffic
                wg0 = wgu.tile([128, NK * 128], BF16, tag="wg", name="wg0")
                wu0 = wgu.tile([128, NK * 128], BF16, tag="wu", name="wu0")
                # per-k x tiles: fine-grained deps let the first matmuls start
                # as soon as their own h-slice lands, not the whole block
                x_sb = [xp.tile([128, tcols], BF16, tag=f"x{k}", name=f"xsb{k}")
                        for k in range(NK)]
                wgu1 = None
                wgu2 = None
                if bi == 0 and ni > 1:
                    # startup is stream-latency-critical: deliver weight
                    # quarter-tiles just-in-time between x tiles (one quarter
                    # per two x tiles keeps the serial DMA pipe ahead of the
                    # k-outer warmup's ~0.85us/x-tile consumption), and use
                    # the 3rd wgu buffer to prefetch icol 2 behind the warmup
                    q4 = NK * 128 // 4
                    wg1 = wgu.tile([128, NK * 128], BF16, tag="wg", name="wg1")
                    wu1 = wgu.tile([128, NK * 128], BF16, tag="wu", name="wu1")
                    wgu1 = (wg1, wu1)
                    quarters = [(t_, d_, q) for q in range(4)
                                for (t_, d_) in ((wg0, g_d), (wu0, u_d),
                                                 (wg1, g_d), (wu1, u_d))]
                    qi = 0

                    def _q():
                        nonlocal qi
                        t_, d_, q = quarters[qi]
                        it_ = 1 if t_ in (wg1, wu1) else 0
                        qi += 1
                        nc.sync.dma_start(t_[:, q * q4:(q + 1) * q4],
                                          d_.ap()[it_, :, q * q4:(q + 1) * q4])

                    _q()  # wg0 cols 0:1024 (k=0..7)
                    nc.sync.dma_start(x_sb[0][:], xt_d.ap()[0, :, t0:t0 + tcols])
                    _q()  # wu0 q0
                    for k in range(1, NK):
                        nc.sync.dma_start(x_sb[k][:], xt_d.ap()[k, :, t0:t0 + tcols])
                        if k % 2 == 1 and qi < len(quarters):
                            _q()
                    while qi < len(quarters):
                        _q()
                    if ni > 2:
                        wg2 = wgu.tile([128, NK * 128], BF16, tag="wg", name="wg2")
                        wu2 = wgu.tile([128, NK * 128], BF16, tag="wu", name="wu2")
                        wgu2 = (wg2, wu2)
                        for half in range(2):
                            h0 = half * (q4 * 2)
                            nc.sync.dma_start(wg2[:, h0:h0 + q4 * 2],
                                              g_d.ap()[2, :, h0:h0 + q4 * 2])
                            nc.sync.dma_start(wu2[:, h0:h0 + q4 * 2],
                                              u_d.ap()[2, :, h0:h0 + q4 * 2])
                else:
                    nc.sync.dma_start(wg0[:], g_d.ap()[0])
                    nc.sync.dma_start(wu0[:], u_d.ap()[0])
                    for k in range(NK):
                        nc.sync.dma_start(x_sb[k][:], xt_d.ap()[k, :, t0:t0 + tcols])
                a_sb = apool.tile([128, ni, tcols], BF16, tag="a")
                it_start = 0
                if bi == 0 and wgu1 is not None and len(subs) == 1:
                    # warmup: k-outer over icol tiles 0+1 so the PE consumes
                    # each x[k] for ~0.85us as it lands (x stream paces the
                    # start; sub-outer would idle between arrivals)
                    w = subs[0][1]
                    wg1, wu1 = wgu1
                    pg0 = pgp.tile([128, w], FP32, tag="pg")
                    pu0 = pup.tile([128, w], FP32, tag="pu")
                    pg1 = pgp.tile([128, w], FP32, tag="pg")
                    pu1 = pup.tile([128, w], FP32, tag="pu")
                    for k in range(NK):
                        kk = slice(k * 128, (k + 1) * 128)
                        st, sp_ = (k == 0), (k == NK - 1)
                        nc.tensor.matmul(pg0[:], wg0[:, kk], x_sb[k][:],
                                         start=st, stop=sp_)
                        nc.tensor.matmul(pu0[:], wu0[:, kk], x_sb[k][:],
                                         start=st, stop=sp_)
                        nc.tensor.matmul(pg1[:], wg1[:, kk], x_sb[k][:],
                                         start=st, stop=sp_)
                        nc.tensor.matmul(pu1[:], wu1[:, kk], x_sb[k][:],
                                         start=st, stop=sp_)
                    for it_, pg_, pu_ in ((0, pg0, pu0), (1, pg1, pu1)):
                        silu_sb = sp.tile([128, w], FP32, tag="silu")
                        nc.scalar.activation(silu_sb[:], pg_[:],
                                             mybir.ActivationFunctionType.Silu)
                        nc.vector.tensor_mul(a_sb[:, it_, :], silu_sb[:], pu_[:])
                    it_start = 2
                for it in range(it_start, ni):
                    if it == 0:
                        wg_sb, wu_sb = wg0, wu0
                    elif it == 1 and wgu1 is not None:
                        wg_sb, wu_sb = wgu1
                    elif it == 2 and wgu2 is not None:
                        wg_sb, wu_sb = wgu2
                    else:
                        wg_sb = wgu.tile([128, NK * 128], BF16, tag="wg")
                        wu_sb = wgu.tile([128, NK * 128], BF16, tag="wu")
                        nc.sync.dma_start(wg_sb[:], g_d.ap()[it])
                        nc.sync.dma_start(wu_sb[:], u_d.ap()[it])
                    for (c0, w) in subs:
                        c1 = c0 + w
                        pg = pgp.tile([128, w], FP32, tag="pg")
                        pu = pup.tile([128, w], FP32, tag="pu")
                        for k in range(NK):
                            nc.tensor.matmul(pg[:], wg_sb[:, k * 128:(k + 1) * 128],
                                             x_sb[k][:, c0:c1],
                                             start=(k == 0), stop=(k == NK - 1))
                        for k in range(NK):
                            nc.tensor.matmul(pu[:], wu_sb[:, k * 128:(k + 1) * 128],
                                             x_sb[k][:, c0:c1],
                                             start=(k == 0), stop=(k == NK - 1))
                        silu_sb = sp.tile([128, w], FP32, tag="silu")
                        nc.scalar.activation(silu_sb[:], pg[:],
                                             mybir.ActivationFunctionType.Silu)
                        nc.vector.tensor_mul(a_sb[:, it, c0:c1], silu_sb[:], pu[:])
                for h in range(NK):
                    wd_sb = wdp.tile([128, ni * 128], BF16, tag="wd")
                    nc.sync.dma_start(wd_sb[:], d_d.ap()[h])
                    subs_h = subs
                    if bi == len(blocks) - 1 and h == NK - 1 and subs[-1][1] > 160:
                        # end on a narrow sub so the final PSUM->SBUF->HBM
                        # drain after the last matmul covers few columns
                        lc0, lw = subs[-1]
                        subs_h = subs[:-1] + [(lc0, lw - 128), (lc0 + lw - 128, 128)]
                    for (c0, w) in subs_h:
                        c1 = c0 + w
                        py = pyp.tile([128, w], FP32, tag="py")
                        for i in range(ni):
                            nc.tensor.matmul(py[:], wd_sb[:, i * 128:(i + 1) * 128],
                                             a_sb[:, i, c0:c1],
                                             start=(i == 0), stop=(i == ni - 1))
                        y_sb = yp.tile([128, w], FP32, tag="y")
                        nc.scalar.copy(y_sb[:], py[:])
                        nc.sync.dma_start(yt_d.ap()[h, :, t0 + c0:t0 + c1], y_sb[:])

    nc.compile()
    nc.m = get_hw_module(nc.m)
    return nc


def _build_tp8(nl, nv):
    blocks = [("l", t0, tc) for (t0, tc) in _plan_blocks(nl, 1024)]
    blocks += [("v", nl + t0, tc) for (t0, tc) in _plan_blocks(nv, 1024)]
    return _build(nl + nv, NI8,
                  {"l": ("gl", "ul", "dl"), "v": ("gv", "uv", "dv")},
                  blocks, a_bufs=2, wd_bufs=3, y_bufs=4)


def _build_aff(cap):
    # blocks of ~512 keep the [128, 43, tcols] "a" tile within SBUF
    blocks = [("e", t0, tc) for (t0, tc) in _plan_blocks(cap, 512)]
    return _build(cap, NI2, {"e": ("g", "u", "d")},
                  blocks, a_bufs=2, wd_bufs=2, y_bufs=2)


def _tile_gu(W, c, ish, ni):
    """[H, I] f32 -> per-core [ni, 128, NK*128] bf16 column shard."""
    sh = np.asarray(W, dtype=np.float32)[:, c * ish:(c + 1) * ish].astype(bf16)
    pad = ni * 128 - ish
    if pad:
        sh = np.concatenate([sh, np.zeros((H, pad), dtype=bf16)], axis=1)
    t = sh.reshape(NK, 128, ni, 128).transpose(2, 1, 0, 3)
    return np.ascontiguousarray(t).reshape(ni, 128, NK * 128)


def _tile_d(W, c, ish, ni):
    """[I, H] f32 -> per-core [NK, 128, ni*128] bf16 row shard."""
    sh = np.asarray(W, dtype=np.float32)[c * ish:(c + 1) * ish, :].astype(bf16)
    pad = ni * 128 - ish
    if pad:
        sh = np.concatenate([sh, np.zeros((pad, H), dtype=bf16)], axis=0)
    t = sh.reshape(ni, 128, NK, 128).transpose(2, 1, 0, 3)
    return np.ascontiguousarray(t).reshape(NK, 128, ni * 128)


def _chunks(n, k):
    if k <= 0:
        return []
    base, rem = divmod(n, k)
    out, s = [], 0
    for i in range(k):
        c = base + (1 if i < rem else 0)
        out.append((s, c))
        s += c
    return out


def _affinity_shards(Nl, Nv):
    """4 single-expert token shards for the DP4 x TP2 layout, or None if the
    expert split is too skewed for this to beat TP8."""
    if Nl == 0 or Nv == 0:
        k_l = 4 if Nv == 0 else 0
    else:
        k_l = min(3, max(1, round(4 * Nl / (Nl + Nv))))
    shards = ([("l", s, c) for (s, c) in _chunks(Nl, k_l)]
              + [("v", s, c) for (s, c) in _chunks(Nv, 4 - k_l)])
    if len(shards) != 4 or any(c == 0 for _, _, c in shards):
        return None, 0
    cap = max(c for _, _, c in shards)
    # affinity wins only while its per-core work (cap x 5504 exact icols)
    # undercuts TP8's (all tokens x 1408 padded icols)
    if cap * ISH2 >= (Nl + Nv) * NI8 * 128:
        return None, 0
    return shards, cap


def kernel(hidden_states, token_type_ids, lang_gate, lang_up, lang_down,
           vis_gate, vis_up, vis_down):
    global last_results, last_run
    x = np.asarray(hidden_states, dtype=np.float32).reshape(B * S, H)
    tt = np.asarray(token_type_ids).reshape(B, S)

    vis = np.zeros((B, S), dtype=bool)
    vis[:, :-1] = (tt[:, :-1] == VISION_TOKEN_TYPE) & (tt[:, 1:] == VISION_TOKEN_TYPE)
    visf = vis.reshape(-1)
    lang_idx = np.flatnonzero(~visf)
    vis_idx = np.flatnonzero(visf)
    Nl, Nv = len(lang_idx), len(vis_idx)
    ew = {"l": (lang_gate, lang_up, lang_down), "v": (vis_gate, vis_up, vis_down)}

    shards, cap = _affinity_shards(Nl, Nv)
    if shards is not None:
        # ---- expert-affinity DP4 x TP2 ----
        key = ("aff", cap)
        if key not in _nc_cache:
            _nc_cache[key] = _build_aff(cap)
        nc = _nc_cache[key]

        wt = {}  # (expert, tp) -> tiled weights
        for e in set(e for e, _, _ in shards):
            g, u, d = ew[e]
            for tp in range(2):
                wt[(e, tp)] = (_tile_gu(g, tp, ISH2, NI2),
                               _tile_gu(u, tp, ISH2, NI2),
                               _tile_d(d, tp, ISH2, NI2))
        in_maps = [None] * NCORES
        shard_idx = []
        for s, (e, st, cnt) in enumerate(shards):
            idx = (lang_idx if e == "l" else vis_idx)[st:st + cnt]
            shard_idx.append(idx)
            xs = np.zeros((cap, H), dtype=np.float32)
            xs[:cnt] = x[idx]
            xt_s = np.ascontiguousarray(xs.T.astype(bf16)).reshape(NK, 128, cap)
            for tp in range(2):
                g_t, u_t, d_t = wt[(e, tp)]
                in_maps[2 * s + tp] = {"xt": xt_s, "g": g_t, "u": u_t, "d": d_t}

        trace = bool(int(os.environ.get("KERNEL_TRACE", "0")))
        res = run_bass_kernel_spmd(nc, in_maps, list(range(NCORES)), trace=trace)
        last_results = res
        last_run = (nc, in_maps)

        out_flat = np.empty((B * S, H), dtype=np.float32)
        for s, (e, st, cnt) in enumerate(shards):
            ysum = (res.results[2 * s]["yt"] + res.results[2 * s + 1]["yt"])
            out_flat[shard_idx[s]] = ysum.reshape(H, cap)[:, :cnt].T
        return out_flat.reshape(B, S, H)

    # ---- TP8 fallback: shard I 8 ways, every core runs all tokens ----
    Tt = Nl + Nv
    xp_ = np.empty((Tt, H), dtype=np.float32)
    xp_[:Nl] = x[lang_idx]
    xp_[Nl:] = x[vis_idx]
    xt = np.ascontiguousarray(xp_.T.astype(bf16)).reshape(NK, 128, Tt)

    key = ("tp8", Nl, Nv)
    if key not in _nc_cache:
        _nc_cache[key] = _build_tp8(Nl, Nv)
    nc = _nc_cache[key]

    in_maps = []
    for c in range(NCORES):
        in_maps.append({
            "xt": xt,
            "gl": _tile_gu(lang_gate, c, ISH8, NI8),
            "ul": _tile_gu(lang_up, c, ISH8, NI8),
            "dl": _tile_d(lang_down, c, ISH8, NI8),
            "gv": _tile_gu(vis_gate, c, ISH8, NI8),
            "uv": _tile_gu(vis_up, c, ISH8, NI8),
            "dv": _tile_d(vis_down, c, ISH8, NI8),
        })

    trace = bool(int(os.environ.get("KERNEL_TRACE", "0")))
    res = run_bass_kernel_spmd(nc, in_maps, list(range(NCORES)), trace=trace)
    last_results = res
    last_run = (nc, in_maps)

    ysum = np.zeros((NK, 128, Tt), dtype=np.float32)
    for r in res.results:
        ysum += r["yt"]
    yt_full = ysum.reshape(H, Tt)
    out_flat = np.empty((B * S, H), dtype=np.float32)
    out_flat[lang_idx] = yt_full[:, :Nl].T
    out_flat[vis_idx] = yt_full[:, Nl:].T
    return out_flat.reshape(B, S, H)



# revision 3
# speedup vs baseline: 1.0450x; 1.0450x over previous
"""CogVLM VisionExpert MLP (moe routing) on 8 trn2 NeuronCores.

Strategy:
  - Host computes the vision/language routing mask in numpy and permutes
    tokens by expert. Each token goes through exactly ONE expert (the
    reference computes both and selects; routing halves the matmul work).
  - Default sharding (expert-affinity DP4 x TP2): four 2-core tensor-parallel
    pairs, each pair owns one expert and a contiguous slice of that expert's
    tokens (for this problem's ~3:1 lang:vis split: 3 lang pairs + 1 vis
    pair). TP halves split the intermediate dim I=11008 -> 5504 = 43*128
    exactly, so the icol padding TP8 needs (1376->1408, +2.3% matmuls)
    vanishes, and every token block is >=~512 wide so the per-block weight
    stream hides under the matmuls. Host sums each pair's two partial
    outputs (the "all-reduce") and un-permutes.
  - Fallback (TP8 over I, all tokens on every core) when the expert split is
    too skewed for affinity pairs to balance.
  - bf16 matmuls with fp32 PSUM accumulation; no token padding anywhere (the
    matmul moving dim accepts any width <= 512; ragged blocks use balanced
    sub-widths so no tiny trailing matmuls).

Layouts shipped to the device (everything lands in SBUF with natural
[128-partition, free] shape and fully-contiguous DMA runs):
  xt   [NK, 128, T]       bf16   xt[k, p, t]       = X_perm[t, k*128+p]
  g/u  [ni, 128, NK*128]  bf16   g[it, p, k*128+c] = Wg_sh[k*128+p, it*128+c]
  d    [NK, 128, ni*128]  bf16   d[h, p, i*128+c]  = Wd_sh[i*128+p, h*128+c]
  yt   [NK, 128, T]       f32    yt[h, p, t]       = Y_part[t, h*128+p]
"""
import os
import numpy as np
import ml_dtypes

import concourse.bacc as bacc
import concourse.mybir as mybir
import concourse.tile as tile
from concourse.bass_utils import run_bass_kernel_spmd
from concourse.bass_interp import get_hw_module

bf16 = ml_dtypes.bfloat16
B, S, H, I = 2, 2048, 4096, 11008
NCORES = 8
NK = H // 128                # 32 h tiles
TSUB = 512                   # matmul moving-dim chunk (one PSUM bank of f32)
VISION_TOKEN_TYPE = 1

# TP8 fallback geometry
ISH8 = I // 8                # 1376 real icols per core
NI8 = (ISH8 + 127) // 128    # 11 tiles (padded to 1408)
# expert-affinity DP4 x TP2 geometry
ISH2 = I // 2                # 5504 icols per core
NI2 = ISH2 // 128            # 43 tiles, exact

FP32 = mybir.dt.float32
BF16 = mybir.dt.bfloat16

_nc_cache = {}

# observability for test harnesses (not used by grading)
last_results = None
last_run = None  # (nc, in_maps) of the most recent kernel() call


def _plan_blocks(n, tblk):
    """Split a token count into blocks of <= tblk+128 (weights re-stream once
    per block). A remainder <=128 is folded into the last block: a standalone
    narrow block starves the PE on its weight stream."""
    out = []
    t = 0
    while n - t >= tblk:
        out.append((t, tblk))
        t += tblk
    r = n - t
    if r > 0:
        if r <= 128 and out:
            t0, tc = out[-1]
            out[-1] = (t0, tc + r)
        else:
            out.append((t, r))
    return out


def _subs(tcols):
    """Split a block into balanced matmul moving-dim pieces (<= 512 each),
    e.g. 1057 -> 353/352/352 — avoids tiny trailing matmuls whose LDWEIGHTS
    cost can't hide under the streaming."""
    nsub = (tcols + TSUB - 1) // TSUB
    base, rem = divmod(tcols, nsub)
    out = []
    c = 0
    for s in range(nsub):
        w = base + (1 if s < rem else 0)
        out.append((c, w))
        c += w
    return out


def _build(Tt, ni, weight_sets, blocks, a_bufs, wd_bufs, y_bufs):
    """Emit the SPMD program: for each (set_key, t0, tcols) block run the
    swiglu MLP over that token window with that weight set's shards."""
    nc = bacc.Bacc("TRN2", target_bir_lowering=False, debug=False,
                   num_devices=NCORES)

    xt_d = nc.dram_tensor("xt", [NK, 128, Tt], BF16, kind="ExternalInput")
    w_d = {}
    for key, (gn, un, dn) in weight_sets.items():
        w_d[key] = (
            nc.dram_tensor(gn, [ni, 128, NK * 128], BF16, kind="ExternalInput"),
            nc.dram_tensor(un, [ni, 128, NK * 128], BF16, kind="ExternalInput"),
            nc.dram_tensor(dn, [NK, 128, ni * 128], BF16, kind="ExternalInput"),
        )
    yt_d = nc.dram_tensor("yt", [NK, 128, Tt], FP32, kind="ExternalOutput")

    with tile.TileContext(nc) as tc_:
        with (
            tc_.tile_pool(name="xp", bufs=1) as xp,
            tc_.tile_pool(name="apool", bufs=a_bufs) as apool,
            tc_.tile_pool(name="wgu", bufs=3) as wgu,
            tc_.tile_pool(name="wdp", bufs=wd_bufs) as wdp,
            tc_.tile_pool(name="sp", bufs=2) as sp,
            tc_.tile_pool(name="yp", bufs=y_bufs) as yp,
            tc_.tile_pool(name="pg", bufs=2, space="PSUM") as pgp,
            tc_.tile_pool(name="pu", bufs=2, space="PSUM") as pup,
            tc_.tile_pool(name="py", bufs=4, space="PSUM") as pyp,
        ):
            for bi, (key, t0, tcols) in enumerate(blocks):
                g_d, u_d, d_d = w_d[key]
                subs = _subs(tcols)
                # issue the first gate/up weight DMAs before the x block so
                # the opening matmuls aren't queued behind the x traffic
                wg0 = wgu.tile([128, NK * 128], BF16, tag="wg", name="wg0")
                wu0 = wgu.tile([128, NK * 128], BF16, tag="wu", name="wu0")
                # per-k x tiles: fine-grained deps let the first matmuls start
                # as soon as their own h-slice lands, not the whole block
                x_sb = [xp.tile([128, tcols], BF16, tag=f"x{k}", name=f"xsb{k}")
                        for k in range(NK)]
                wgu1 = None
                wgu2 = None
                if bi == 0 and ni > 1:
                    # startup is stream-latency-critical: deliver weight
                    # quarter-tiles just-in-time between x tiles (one quarter
                    # per two x tiles keeps the serial DMA pipe ahead of the
                    # k-outer warmup's ~0.85us/x-tile consumption), and use
                    # the 3rd wgu buffer to prefetch icol 2 behind the warmup
                    q4 = NK * 128 // 4
                    wg1 = wgu.tile([128, NK * 128], BF16, tag="wg", name="wg1")
                    wu1 = wgu.tile([128, NK * 128], BF16, tag="wu", name="wu1")
                    wgu1 = (wg1, wu1)
                    quarters = [(t_, d_, q) for q in range(4)
                                for (t_, d_) in ((wg0, g_d), (wu0, u_d),
                                                 (wg1, g_d), (wu1, u_d))]
                    qi = 0

                    def _q():
                        nonlocal qi
                        t_, d_, q = quarters[qi]
                        it_ = 1 if t_ in (wg1, wu1) else 0
                        qi += 1
                        nc.sync.dma_start(t_[:, q * q4:(q + 1) * q4],
                                          d_.ap()[it_, :, q * q4:(q + 1) * q4])

                    _q()  # wg0 cols 0:1024 (k=0..7)
                    nc.sync.dma_start(x_sb[0][:], xt_d.ap()[0, :, t0:t0 + tcols])
                    _q()  # wu0 q0
                    for k in range(1, NK):
                        nc.sync.dma_start(x_sb[k][:], xt_d.ap()[k, :, t0:t0 + tcols])
                        if k % 2 == 1 and qi < len(quarters):
                            _q()
                    while qi < len(quarters):
                        _q()
                    if ni > 2:
                        wg2 = wgu.tile([128, NK * 128], BF16, tag="wg", name="wg2")
                        wu2 = wgu.tile([128, NK * 128], BF16, tag="wu", name="wu2")
                        wgu2 = (wg2, wu2)
                        for half in range(2):
                            h0 = half * (q4 * 2)
                            nc.sync.dma_start(wg2[:, h0:h0 + q4 * 2],
                                              g_d.ap()[2, :, h0:h0 + q4 * 2])
                            nc.sync.dma_start(wu2[:, h0:h0 + q4 * 2],
                                              u_d.ap()[2, :, h0:h0 + q4 * 2])
                else:
                    nc.sync.dma_start(wg0[:], g_d.ap()[0])
                    nc.sync.dma_start(wu0[:], u_d.ap()[0])
                    for k in range(NK):
                        nc.sync.dma_start(x_sb[k][:], xt_d.ap()[k, :, t0:t0 + tcols])
                a_sb = apool.tile([128, ni, tcols], BF16, tag="a")
                it_start = 0
                if bi == 0 and wgu1 is not None and len(subs) == 1:
                    # warmup: k-outer over icol tiles 0+1 so the PE consumes
                    # each x[k] for ~0.85us as it lands (x stream paces the
                    # start; sub-outer would idle between arrivals)
                    w = subs[0][1]
                    wg1, wu1 = wgu1
                    pg0 = pgp.tile([128, w], FP32, tag="pg")
                    pu0 = pup.tile([128, w], FP32, tag="pu")
                    pg1 = pgp.tile([128, w], FP32, tag="pg")
                    pu1 = pup.tile([128, w], FP32, tag="pu")
                    for k in range(NK):
                        kk = slice(k * 128, (k + 1) * 128)
                        st, sp_ = (k == 0), (k == NK - 1)
                        nc.tensor.matmul(pg0[:], wg0[:, kk], x_sb[k][:],
                                         start=st, stop=sp_)
                        nc.tensor.matmul(pu0[:], wu0[:, kk], x_sb[k][:],
                                         start=st, stop=sp_)
                        nc.tensor.matmul(pg1[:], wg1[:, kk], x_sb[k][:],
                                         start=st, stop=sp_)
                        nc.tensor.matmul(pu1[:], wu1[:, kk], x_sb[k][:],
                                         start=st, stop=sp_)
                    for it_, pg_, pu_ in ((0, pg0, pu0), (1, pg1, pu1)):
                        silu_sb = sp.tile([128, w], FP32, tag="silu")
                        nc.scalar.activation(silu_sb[:], pg_[:],
                                             mybir.ActivationFunctionType.Silu)
                        nc.vector.tensor_mul(a_sb[:, it_, :], silu_sb[:], pu_[:])
                    it_start = 2
                for it in range(it_start, ni):
                    if it == 0:
                        wg_sb, wu_sb = wg0, wu0
                    elif it == 1 and wgu1 is not None:
                        wg_sb, wu_sb = wgu1
                    elif it == 2 and wgu2 is not None:
                        wg_sb, wu_sb = wgu2
                    else:
                        wg_sb = wgu.tile([128, NK * 128], BF16, tag="wg")
                        wu_sb = wgu.tile([128, NK * 128], BF16, tag="wu")
                        nc.sync.dma_start(wg_sb[:], g_d.ap()[it])
                        nc.sync.dma_start(wu_sb[:], u_d.ap()[it])
                    for (c0, w) in subs:
                        c1 = c0 + w
                        pg = pgp.tile([128, w], FP32, tag="pg")
                        pu = pup.tile([128, w], FP32, tag="pu")
                        for k in range(NK):
                            nc.tensor.matmul(pg[:], wg_sb[:, k * 128:(k + 1) * 128],
                                             x_sb[k][:, c0:c1],
                                             start=(k == 0), stop=(k == NK - 1))
                        for k in range(NK):
                            nc.tensor.matmul(pu[:], wu_sb[:, k * 128:(k + 1) * 128],
                                             x_sb[k][:, c0:c1],
                                             start=(k == 0), stop=(k == NK - 1))
                        silu_sb = sp.tile([128, w], FP32, tag="silu")
                        nc.scalar.activation(silu_sb[:], pg[:],
                                             mybir.ActivationFunctionType.Silu)
                        nc.vector.tensor_mul(a_sb[:, it, c0:c1], silu_sb[:], pu[:])
                for h in range(NK):
                    wd_sb = wdp.tile([128, ni * 128], BF16, tag="wd")
                    nc.sync.dma_start(wd_sb[:], d_d.ap()[h])
                    subs_h = subs
                    if bi == len(blocks) - 1 and h == NK - 1 and subs[-1][1] > 160:
                        # end on a narrow sub so the final PSUM->SBUF->HBM
                        # drain after the last matmul covers few columns
                        lc0, lw = subs[-1]
                        subs_h = subs[:-1] + [(lc0, lw - 128), (lc0 + lw - 128, 128)]
                    for (c0, w) in subs_h:
                        c1 = c0 + w
                        py = pyp.tile([128, w], FP32, tag="py")
                        for i in range(ni):
                            nc.tensor.matmul(py[:], wd_sb[:, i * 128:(i + 1) * 128],
                                             a_sb[:, i, c0:c1],
                                             start=(i == 0), stop=(i == ni - 1))
                        y_sb = yp.tile([128, w], FP32, tag="y")
                        nc.scalar.copy(y_sb[:], py[:])
                        nc.sync.dma_start(yt_d.ap()[h, :, t0 + c0:t0 + c1], y_sb[:])

    nc.compile()
    nc.m = get_hw_module(nc.m)
    return nc


def _build_tp8(nl, nv):
    blocks = [("l", t0, tc) for (t0, tc) in _plan_blocks(nl, 1024)]
    blocks += [("v", nl + t0, tc) for (t0, tc) in _plan_blocks(nv, 1024)]
    return _build(nl + nv, NI8,
                  {"l": ("gl", "ul", "dl"), "v": ("gv", "uv", "dv")},
                  blocks, a_bufs=2, wd_bufs=3, y_bufs=4)


def _build_slots(B0, B1):
    """Uniform two-weight-slot program: every core runs block0 (B0 tokens,
    weight set 0) then block1 (B1 tokens, weight set 1). Which expert each
    slot holds is decided per core purely by the input tensors."""
    blocks = [("w0", 0, B0), ("w1", B0, B1)]
    return _build(B0 + B1, NI2,
                  {"w0": ("g0", "u0", "d0"), "w1": ("g1", "u1", "d1")},
                  blocks, a_bufs=1, wd_bufs=3, y_bufs=2)


BMIN = 256  # min block tokens: a 135MB TP2 weight stream hides under ~>=220


def _plan_slots(Nl, Nv):
    """Pick (B0, B1, jv0, jv1): vis occupies jv0 slot-0s and jv1 slot-1s,
    lang the rest; minimize per-core tokens T = B0 + B1."""
    best = None
    for jv0 in range(5):
        for jv1 in range(5):
            jl0, jl1 = 4 - jv0, 4 - jv1
            for B0 in range(BMIN, 1101):
                rv = Nv - jv0 * B0
                rl = Nl - jl0 * B0
                if jv1 == 0 and rv > 0:
                    continue
                if jl1 == 0 and rl > 0:
                    continue
                B1 = BMIN
                if jv1 > 0:
                    B1 = max(B1, -(-rv // jv1))
                if jl1 > 0:
                    B1 = max(B1, -(-rl // jl1))
                # prefer balanced blocks (bigger min-block -> deeper weight
                # stream hiding), then block0 <= block1 (warmup needs <=512)
                cand = (B0 + B1, -min(B0, B1), B0 > B1, B0, B1, jv0, jv1)
                if best is None or cand < best:
                    best = cand
    if best is None:
        return None
    B0, B1, jv0, jv1 = best[-4:]
    return B0, B1, jv0, jv1


def _tile_gu(W, c, ish, ni):
    """[H, I] f32 -> per-core [ni, 128, NK*128] bf16 column shard."""
    sh = np.asarray(W, dtype=np.float32)[:, c * ish:(c + 1) * ish].astype(bf16)
    pad = ni * 128 - ish
    if pad:
        sh = np.concatenate([sh, np.zeros((H, pad), dtype=bf16)], axis=1)
    t = sh.reshape(NK, 128, ni, 128).transpose(2, 1, 0, 3)
    return np.ascontiguousarray(t).reshape(ni, 128, NK * 128)


def _tile_d(W, c, ish, ni):
    """[I, H] f32 -> per-core [NK, 128, ni*128] bf16 row shard."""
    sh = np.asarray(W, dtype=np.float32)[c * ish:(c + 1) * ish, :].astype(bf16)
    pad = ni * 128 - ish
    if pad:
        sh = np.concatenate([sh, np.zeros((pad, H), dtype=bf16)], axis=0)
    t = sh.reshape(ni, 128, NK, 128).transpose(2, 1, 0, 3)
    return np.ascontiguousarray(t).reshape(NK, 128, ni * 128)


def _chunks(n, k):
    if k <= 0:
        return []
    base, rem = divmod(n, k)
    out, s = [], 0
    for i in range(k):
        c = base + (1 if i < rem else 0)
        out.append((s, c))
        s += c
    return out


def _affinity_shards(Nl, Nv):
    """4 single-expert token shards for the DP4 x TP2 layout, or None if the
    expert split is too skewed for this to beat TP8."""
    if Nl == 0 or Nv == 0:
        k_l = 4 if Nv == 0 else 0
    else:
        k_l = min(3, max(1, round(4 * Nl / (Nl + Nv))))
    shards = ([("l", s, c) for (s, c) in _chunks(Nl, k_l)]
              + [("v", s, c) for (s, c) in _chunks(Nv, 4 - k_l)])
    if len(shards) != 4 or any(c == 0 for _, _, c in shards):
        return None, 0
    cap = max(c for _, _, c in shards)
    # affinity wins only while its per-core work (cap x 5504 exact icols)
    # undercuts TP8's (all tokens x 1408 padded icols)
    if cap * ISH2 >= (Nl + Nv) * NI8 * 128:
        return None, 0
    return shards, cap


def kernel(hidden_states, token_type_ids, lang_gate, lang_up, lang_down,
           vis_gate, vis_up, vis_down):
    global last_results, last_run
    x = np.asarray(hidden_states, dtype=np.float32).reshape(B * S, H)
    tt = np.asarray(token_type_ids).reshape(B, S)

    vis = np.zeros((B, S), dtype=bool)
    vis[:, :-1] = (tt[:, :-1] == VISION_TOKEN_TYPE) & (tt[:, 1:] == VISION_TOKEN_TYPE)
    visf = vis.reshape(-1)
    lang_idx = np.flatnonzero(~visf)
    vis_idx = np.flatnonzero(visf)
    Nl, Nv = len(lang_idx), len(vis_idx)
    ew = {"l": (lang_gate, lang_up, lang_down), "v": (vis_gate, vis_up, vis_down)}

    B0, B1, jv0, jv1 = _plan_slots(Nl, Nv)
    cap = B0 + B1
    # vis takes the LAST jv0 pairs' slot0 / jv1 pairs' slot1
    slot_expert = [["v" if (s == 0 and p >= 4 - jv0) or (s == 1 and p >= 4 - jv1)
                    else "l" for s in range(2)] for p in range(4)]

    key = ("slots", B0, B1)
    if key not in _nc_cache:
        _nc_cache[key] = _build_slots(B0, B1)
    nc = _nc_cache[key]

    wt = {}  # (expert, tp) -> tiled TP2 weight shards
    for e in set(e for row in slot_expert for e in row):
        g, u, d = ew[e]
        for tp in range(2):
            wt[(e, tp)] = (_tile_gu(g, tp, ISH2, NI2),
                           _tile_gu(u, tp, ISH2, NI2),
                           _tile_d(d, tp, ISH2, NI2))

    # sequential fill of each expert's tokens over its slots, pads at tails
    pos = {"l": 0, "v": 0}
    idx_of = {"l": lang_idx, "v": vis_idx}
    slot_tokens = [[None, None] for _ in range(4)]
    for p in range(4):
        for s, bs in ((0, B0), (1, B1)):
            e = slot_expert[p][s]
            take = min(bs, len(idx_of[e]) - pos[e])
            slot_tokens[p][s] = idx_of[e][pos[e]:pos[e] + take]
            pos[e] += take
    assert pos["l"] == Nl and pos["v"] == Nv, (pos, Nl, Nv)

    in_maps = [None] * NCORES
    for p in range(4):
        xs = np.zeros((cap, H), dtype=np.float32)
        i0 = slot_tokens[p][0]
        i1 = slot_tokens[p][1]
        xs[:len(i0)] = x[i0]
        xs[B0:B0 + len(i1)] = x[i1]
        xt_s = np.ascontiguousarray(xs.T.astype(bf16)).reshape(NK, 128, cap)
        for tp in range(2):
            g0, u0, d0 = wt[(slot_expert[p][0], tp)]
            g1, u1, d1 = wt[(slot_expert[p][1], tp)]
            in_maps[2 * p + tp] = {"xt": xt_s, "g0": g0, "u0": u0, "d0": d0,
                                   "g1": g1, "u1": u1, "d1": d1}

    trace = bool(int(os.environ.get("KERNEL_TRACE", "0")))
    res = run_bass_kernel_spmd(nc, in_maps, list(range(NCORES)), trace=trace)
    last_results = res
    last_run = (nc, in_maps)

    out_flat = np.empty((B * S, H), dtype=np.float32)
    for p in range(4):
        ysum = (res.results[2 * p]["yt"] + res.results[2 * p + 1]["yt"])
        yt = ysum.reshape(H, cap)
        out_flat[slot_tokens[p][0]] = yt[:, :len(slot_tokens[p][0])].T
        out_flat[slot_tokens[p][1]] = yt[:, B0:B0 + len(slot_tokens[p][1])].T
    return out_flat.reshape(B, S, H)



# revision 4
# speedup vs baseline: 1.2207x; 1.1682x over previous
"""CogVLM VisionExpert MLP (moe routing) on 8 trn2 NeuronCores.

Strategy:
  - Host computes the vision/language routing mask in numpy and permutes
    tokens by expert. Each token goes through exactly ONE expert (the
    reference computes both and selects; routing halves the matmul work).
  - Uniform two-weight-slot program over 4 TP2 pairs: every core runs the
    SAME instruction stream of two single-expert token blocks (B0, B1
    tokens), but which expert each block computes is decided per core purely
    by the weight tensors fed to that slot. This decouples load balance from
    the expert split: a solver picks (B0, B1) and the vis/lang slot
    assignment minimizing per-core tokens T = B0 + B1 (LP bound ceil((Nl +
    Nv)/4); e.g. 3105/991 -> T = 1025 vs 1035 for pure expert-affinity
    pairs). Pure pairs pass the same expert's weights for both slots.
  - TP2 halves split I = 11008 -> 5504 = 43*128 exactly (no icol padding);
    host sums each pair's two partial outputs and un-permutes. Each block's
    135 MB weight stream hides under >= ~850us of that block's matmuls.
  - Block0 starts with a k-outer warmup over icol tiles 0+1 (PE consumes
    each x[k] tile for ~0.85us as it lands -> the serial DMA pipe, not the
    PE, paces startup), weight quarter-tiles are interleaved into the x
    stream just-in-time, and the last block ends on a narrow 128-col sub so
    the final PSUM drain is short.
  - bf16 matmuls with fp32 PSUM accumulation.

Layouts shipped to the device (everything lands in SBUF with natural
[128-partition, free] shape and fully-contiguous DMA runs):
  xt    [NK, 128, T]       bf16   xt[k, p, t]       = X_perm[t, k*128+p]
  g*/u* [ni, 128, NK*128]  bf16   g[it, p, k*128+c] = Wg_sh[k*128+p, it*128+c]
  d*    [NK, 128, ni*128]  bf16   d[h, p, i*128+c]  = Wd_sh[i*128+p, h*128+c]
  yt    [NK, 128, T]       f32    yt[h, p, t]       = Y_part[t, h*128+p]
"""
import os
import numpy as np
import ml_dtypes

import concourse.bacc as bacc
import concourse.mybir as mybir
import concourse.tile as tile
from concourse.bass_utils import run_bass_kernel_spmd
from concourse.bass_interp import get_hw_module

bf16 = ml_dtypes.bfloat16
B, S, H, I = 2, 2048, 4096, 11008
NCORES = 8
NK = H // 128                # 32 h tiles
TSUB = 512                   # matmul moving-dim chunk (one PSUM bank of f32)
VISION_TOKEN_TYPE = 1

# TP8 fallback geometry
ISH8 = I // 8                # 1376 real icols per core
NI8 = (ISH8 + 127) // 128    # 11 tiles (padded to 1408)
# expert-affinity DP4 x TP2 geometry
ISH2 = I // 2                # 5504 icols per core
NI2 = ISH2 // 128            # 43 tiles, exact

FP32 = mybir.dt.float32
BF16 = mybir.dt.bfloat16

_nc_cache = {}

# observability for test harnesses (not used by grading)
last_results = None
last_run = None  # (nc, in_maps) of the most recent kernel() call


def _plan_blocks(n, tblk):
    """Split a token count into blocks of <= tblk+128 (weights re-stream once
    per block). A remainder <=128 is folded into the last block: a standalone
    narrow block starves the PE on its weight stream."""
    out = []
    t = 0
    while n - t >= tblk:
        out.append((t, tblk))
        t += tblk
    r = n - t
    if r > 0:
        if r <= 128 and out:
            t0, tc = out[-1]
            out[-1] = (t0, tc + r)
        else:
            out.append((t, r))
    return out


def _subs(tcols):
    """Split a block into balanced matmul moving-dim pieces (<= 512 each),
    e.g. 1057 -> 353/352/352 — avoids tiny trailing matmuls whose LDWEIGHTS
    cost can't hide under the streaming."""
    nsub = (tcols + TSUB - 1) // TSUB
    base, rem = divmod(tcols, nsub)
    out = []
    c = 0
    for s in range(nsub):
        w = base + (1 if s < rem else 0)
        out.append((c, w))
        c += w
    return out


def _build(Tt, ni, weight_sets, blocks, a_bufs, wd_bufs, y_bufs):
    """Emit the SPMD program: for each (set_key, t0, tcols) block run the
    swiglu MLP over that token window with that weight set's shards."""
    nc = bacc.Bacc("TRN2", target_bir_lowering=False, debug=False,
                   num_devices=NCORES)

    xt_d = nc.dram_tensor("xt", [NK, 128, Tt], BF16, kind="ExternalInput")
    w_d = {}
    for key, (gn, un, dn) in weight_sets.items():
        w_d[key] = (
            nc.dram_tensor(gn, [ni, 128, NK * 128], BF16, kind="ExternalInput"),
            nc.dram_tensor(un, [ni, 128, NK * 128], BF16, kind="ExternalInput"),
            nc.dram_tensor(dn, [NK, 128, ni * 128], BF16, kind="ExternalInput"),
        )
    yt_d = nc.dram_tensor("yt", [NK, 128, Tt], FP32, kind="ExternalOutput")

    with tile.TileContext(nc) as tc_:
        with (
            tc_.tile_pool(name="xp", bufs=1) as xp,
            tc_.tile_pool(name="apool", bufs=a_bufs) as apool,
            tc_.tile_pool(name="wgu", bufs=3) as wgu,
            tc_.tile_pool(name="wdp", bufs=wd_bufs) as wdp,
            tc_.tile_pool(name="sp", bufs=2) as sp,
            tc_.tile_pool(name="yp", bufs=y_bufs) as yp,
            tc_.tile_pool(name="pg", bufs=2, space="PSUM") as pgp,
            tc_.tile_pool(name="pu", bufs=2, space="PSUM") as pup,
            tc_.tile_pool(name="py", bufs=4, space="PSUM") as pyp,
        ):
            for bi, (key, t0, tcols) in enumerate(blocks):
                g_d, u_d, d_d = w_d[key]
                subs = _subs(tcols)
                # issue the first gate/up weight DMAs before the x block so
                # the opening matmuls aren't queued behind the x traffic
                wg0 = wgu.tile([128, NK * 128], BF16, tag="wg", name="wg0")
                wu0 = wgu.tile([128, NK * 128], BF16, tag="wu", name="wu0")
                # per-k x tiles: fine-grained deps let the first matmuls start
                # as soon as their own h-slice lands, not the whole block
                x_sb = [xp.tile([128, tcols], BF16, tag=f"x{k}", name=f"xsb{k}")
                        for k in range(NK)]
                wgu1 = None
                wgu2 = None
                if bi == 0 and ni > 1:
                    # startup is stream-latency-critical: deliver weight
                    # quarter-tiles just-in-time between x tiles (one quarter
                    # per two x tiles keeps the serial DMA pipe ahead of the
                    # k-outer warmup's ~0.85us/x-tile consumption), and use
                    # the 3rd wgu buffer to prefetch icol 2 behind the warmup
                    q4 = NK * 128 // 4
                    wg1 = wgu.tile([128, NK * 128], BF16, tag="wg", name="wg1")
                    wu1 = wgu.tile([128, NK * 128], BF16, tag="wu", name="wu1")
                    wgu1 = (wg1, wu1)
                    four = ((wg0, g_d, 0), (wu0, u_d, 0), (wg1, g_d, 1),
                            (wu1, u_d, 1))
                    chunks = [(t_, d_, it_, q * q4, (q + 1) * q4)
                              for q in range(4) for (t_, d_, it_) in four]
                    qi = 0

                    def _q():
                        nonlocal qi
                        t_, d_, it_, c0_, c1_ = chunks[qi]
                        qi += 1
                        nc.sync.dma_start(t_[:, c0_:c1_], d_.ap()[it_, :, c0_:c1_])

                    _q()  # wg0 cols 0:1024 (k=0..7)
                    nc.sync.dma_start(x_sb[0][:], xt_d.ap()[0, :, t0:t0 + tcols])
                    _q()  # wu0 q0
                    for k in range(1, NK):
                        nc.sync.dma_start(x_sb[k][:], xt_d.ap()[k, :, t0:t0 + tcols])
                        if k % 2 == 1 and qi < len(chunks):
                            _q()
                    while qi < len(chunks):
                        _q()
                    if ni > 2:
                        wg2 = wgu.tile([128, NK * 128], BF16, tag="wg", name="wg2")
                        wu2 = wgu.tile([128, NK * 128], BF16, tag="wu", name="wu2")
                        wgu2 = (wg2, wu2)
                        for half in range(2):
                            h0 = half * (q4 * 2)
                            nc.sync.dma_start(wg2[:, h0:h0 + q4 * 2],
                                              g_d.ap()[2, :, h0:h0 + q4 * 2])
                            nc.sync.dma_start(wu2[:, h0:h0 + q4 * 2],
                                              u_d.ap()[2, :, h0:h0 + q4 * 2])
                else:
                    nc.sync.dma_start(wg0[:], g_d.ap()[0])
                    nc.sync.dma_start(wu0[:], u_d.ap()[0])
                    for k in range(NK):
                        nc.sync.dma_start(x_sb[k][:], xt_d.ap()[k, :, t0:t0 + tcols])
                a_sb = apool.tile([128, ni, tcols], BF16, tag="a")
                it_start = 0
                if bi == 0 and wgu1 is not None and len(subs) == 1:
                    # warmup: k-outer over icol tiles 0+1 so the PE consumes
                    # each x[k] for ~0.85us as it lands (x stream paces the
                    # start; sub-outer would idle between arrivals)
                    w = subs[0][1]
                    wg1, wu1 = wgu1
                    pg0 = pgp.tile([128, w], FP32, tag="pg")
                    pu0 = pup.tile([128, w], FP32, tag="pu")
                    pg1 = pgp.tile([128, w], FP32, tag="pg")
                    pu1 = pup.tile([128, w], FP32, tag="pu")
                    for k in range(NK):
                        kk = slice(k * 128, (k + 1) * 128)
                        st, sp_ = (k == 0), (k == NK - 1)
                        nc.tensor.matmul(pg0[:], wg0[:, kk], x_sb[k][:],
                                         start=st, stop=sp_)
                        nc.tensor.matmul(pu0[:], wu0[:, kk], x_sb[k][:],
                                         start=st, stop=sp_)
                        nc.tensor.matmul(pg1[:], wg1[:, kk], x_sb[k][:],
                                         start=st, stop=sp_)
                        nc.tensor.matmul(pu1[:], wu1[:, kk], x_sb[k][:],
                                         start=st, stop=sp_)
                    for it_, pg_, pu_ in ((0, pg0, pu0), (1, pg1, pu1)):
                        silu_sb = sp.tile([128, w], FP32, tag="silu")
                        nc.scalar.activation(silu_sb[:], pg_[:],
                                             mybir.ActivationFunctionType.Silu)
                        nc.vector.tensor_mul(a_sb[:, it_, :], silu_sb[:], pu_[:])
                    it_start = 2
                for it in range(it_start, ni):
                    if it == 0:
                        wg_sb, wu_sb = wg0, wu0
                    elif it == 1 and wgu1 is not None:
                        wg_sb, wu_sb = wgu1
                    elif it == 2 and wgu2 is not None:
                        wg_sb, wu_sb = wgu2
                    else:
                        wg_sb = wgu.tile([128, NK * 128], BF16, tag="wg")
                        wu_sb = wgu.tile([128, NK * 128], BF16, tag="wu")
                        nc.sync.dma_start(wg_sb[:], g_d.ap()[it])
                        nc.sync.dma_start(wu_sb[:], u_d.ap()[it])
                    for (c0, w) in subs:
                        c1 = c0 + w
                        pg = pgp.tile([128, w], FP32, tag="pg")
                        pu = pup.tile([128, w], FP32, tag="pu")
                        for k in range(NK):
                            nc.tensor.matmul(pg[:], wg_sb[:, k * 128:(k + 1) * 128],
                                             x_sb[k][:, c0:c1],
                                             start=(k == 0), stop=(k == NK - 1))
                        for k in range(NK):
                            nc.tensor.matmul(pu[:], wu_sb[:, k * 128:(k + 1) * 128],
                                             x_sb[k][:, c0:c1],
                                             start=(k == 0), stop=(k == NK - 1))
                        silu_sb = sp.tile([128, w], FP32, tag="silu")
                        nc.scalar.activation(silu_sb[:], pg[:],
                                             mybir.ActivationFunctionType.Silu)
                        nc.vector.tensor_mul(a_sb[:, it, c0:c1], silu_sb[:], pu[:])
                for h in range(NK):
                    wd_sb = wdp.tile([128, ni * 128], BF16, tag="wd")
                    nc.sync.dma_start(wd_sb[:], d_d.ap()[h])
                    subs_h = subs
                    if bi == len(blocks) - 1 and h == NK - 1 and subs[-1][1] > 160:
                        # end on a narrow sub so the final PSUM->SBUF->HBM
                        # drain after the last matmul covers few columns
                        lc0, lw = subs[-1]
                        subs_h = subs[:-1] + [(lc0, lw - 128), (lc0 + lw - 128, 128)]
                    for (c0, w) in subs_h:
                        c1 = c0 + w
                        py = pyp.tile([128, w], FP32, tag="py")
                        for i in range(ni):
                            nc.tensor.matmul(py[:], wd_sb[:, i * 128:(i + 1) * 128],
                                             a_sb[:, i, c0:c1],
                                             start=(i == 0), stop=(i == ni - 1))
                        y_sb = yp.tile([128, w], FP32, tag="y")
                        nc.scalar.copy(y_sb[:], py[:])
                        nc.sync.dma_start(yt_d.ap()[h, :, t0 + c0:t0 + c1], y_sb[:])

    nc.compile()
    nc.m = get_hw_module(nc.m)
    return nc


def _build_tp8(nl, nv):
    blocks = [("l", t0, tc) for (t0, tc) in _plan_blocks(nl, 1024)]
    blocks += [("v", nl + t0, tc) for (t0, tc) in _plan_blocks(nv, 1024)]
    return _build(nl + nv, NI8,
                  {"l": ("gl", "ul", "dl"), "v": ("gv", "uv", "dv")},
                  blocks, a_bufs=2, wd_bufs=3, y_bufs=4)


def _build_slots(B0, B1):
    """Uniform two-weight-slot program: every core runs block0 (B0 tokens,
    weight set 0) then block1 (B1 tokens, weight set 1). Which expert each
    slot holds is decided per core purely by the input tensors."""
    blocks = [("w0", 0, B0), ("w1", B0, B1)]
    return _build(B0 + B1, NI2,
                  {"w0": ("g0", "u0", "d0"), "w1": ("g1", "u1", "d1")},
                  blocks, a_bufs=1, wd_bufs=3, y_bufs=2)


BMIN = 256  # min block tokens: a 135MB TP2 weight stream hides under ~>=220


def _plan_slots(Nl, Nv):
    """Pick (B0, B1, jv0, jv1): vis occupies jv0 slot-0s and jv1 slot-1s,
    lang the rest; minimize per-core tokens T = B0 + B1."""
    best = None
    for jv0 in range(5):
        for jv1 in range(5):
            jl0, jl1 = 4 - jv0, 4 - jv1
            for B0 in range(BMIN, 1101):
                rv = Nv - jv0 * B0
                rl = Nl - jl0 * B0
                if jv1 == 0 and rv > 0:
                    continue
                if jl1 == 0 and rl > 0:
                    continue
                B1 = BMIN
                if jv1 > 0:
                    B1 = max(B1, -(-rv // jv1))
                if jl1 > 0:
                    B1 = max(B1, -(-rl // jl1))
                # prefer balanced blocks (bigger min-block -> deeper weight
                # stream hiding), then block0 <= block1 (warmup needs <=512)
                cand = (B0 + B1, -min(B0, B1), B0 > B1, B0, B1, jv0, jv1)
                if best is None or cand < best:
                    best = cand
    if best is None:
        return None
    B0, B1, jv0, jv1 = best[-4:]
    return B0, B1, jv0, jv1


def _tile_gu(W, c, ish, ni):
    """[H, I] f32 -> per-core [ni, 128, NK*128] bf16 column shard."""
    sh = np.asarray(W, dtype=np.float32)[:, c * ish:(c + 1) * ish].astype(bf16)
    pad = ni * 128 - ish
    if pad:
        sh = np.concatenate([sh, np.zeros((H, pad), dtype=bf16)], axis=1)
    t = sh.reshape(NK, 128, ni, 128).transpose(2, 1, 0, 3)
    return np.ascontiguousarray(t).reshape(ni, 128, NK * 128)


def _tile_d(W, c, ish, ni):
    """[I, H] f32 -> per-core [NK, 128, ni*128] bf16 row shard."""
    sh = np.asarray(W, dtype=np.float32)[c * ish:(c + 1) * ish, :].astype(bf16)
    pad = ni * 128 - ish
    if pad:
        sh = np.concatenate([sh, np.zeros((pad, H), dtype=bf16)], axis=0)
    t = sh.reshape(ni, 128, NK, 128).transpose(2, 1, 0, 3)
    return np.ascontiguousarray(t).reshape(NK, 128, ni * 128)


def kernel(hidden_states, token_type_ids, lang_gate, lang_up, lang_down,
           vis_gate, vis_up, vis_down):
    global last_results, last_run
    x = np.asarray(hidden_states, dtype=np.float32).reshape(B * S, H)
    tt = np.asarray(token_type_ids).reshape(B, S)

    vis = np.zeros((B, S), dtype=bool)
    vis[:, :-1] = (tt[:, :-1] == VISION_TOKEN_TYPE) & (tt[:, 1:] == VISION_TOKEN_TYPE)
    visf = vis.reshape(-1)
    lang_idx = np.flatnonzero(~visf)
    vis_idx = np.flatnonzero(visf)
    Nl, Nv = len(lang_idx), len(vis_idx)
    ew = {"l": (lang_gate, lang_up, lang_down), "v": (vis_gate, vis_up, vis_down)}

    B0, B1, jv0, jv1 = _plan_slots(Nl, Nv)
    cap = B0 + B1
    # vis takes the LAST jv0 pairs' slot0 / jv1 pairs' slot1
    slot_expert = [["v" if (s == 0 and p >= 4 - jv0) or (s == 1 and p >= 4 - jv1)
                    else "l" for s in range(2)] for p in range(4)]

    key = ("slots", B0, B1)
    if key not in _nc_cache:
        _nc_cache[key] = _build_slots(B0, B1)
    nc = _nc_cache[key]

    wt = {}  # (expert, tp) -> tiled TP2 weight shards
    for e in set(e for row in slot_expert for e in row):
        g, u, d = ew[e]
        for tp in range(2):
            wt[(e, tp)] = (_tile_gu(g, tp, ISH2, NI2),
                           _tile_gu(u, tp, ISH2, NI2),
                           _tile_d(d, tp, ISH2, NI2))

    # sequential fill of each expert's tokens over its slots, pads at tails
    pos = {"l": 0, "v": 0}
    idx_of = {"l": lang_idx, "v": vis_idx}
    slot_tokens = [[None, None] for _ in range(4)]
    for p in range(4):
        for s, bs in ((0, B0), (1, B1)):
            e = slot_expert[p][s]
            take = min(bs, len(idx_of[e]) - pos[e])
            slot_tokens[p][s] = idx_of[e][pos[e]:pos[e] + take]
            pos[e] += take
    assert pos["l"] == Nl and pos["v"] == Nv, (pos, Nl, Nv)

    in_maps = [None] * NCORES
    for p in range(4):
        xs = np.zeros((cap, H), dtype=np.float32)
        i0 = slot_tokens[p][0]
        i1 = slot_tokens[p][1]
        xs[:len(i0)] = x[i0]
        xs[B0:B0 + len(i1)] = x[i1]
        xt_s = np.ascontiguousarray(xs.T.astype(bf16)).reshape(NK, 128, cap)
        for tp in range(2):
            g0, u0, d0 = wt[(slot_expert[p][0], tp)]
            g1, u1, d1 = wt[(slot_expert[p][1], tp)]
            in_maps[2 * p + tp] = {"xt": xt_s, "g0": g0, "u0": u0, "d0": d0,
                                   "g1": g1, "u1": u1, "d1": d1}

    trace = bool(int(os.environ.get("KERNEL_TRACE", "0")))
    res = run_bass_kernel_spmd(nc, in_maps, list(range(NCORES)), trace=trace)
    last_results = res
    last_run = (nc, in_maps)

    out_flat = np.empty((B * S, H), dtype=np.float32)
    for p in range(4):
        ysum = (res.results[2 * p]["yt"] + res.results[2 * p + 1]["yt"])
        yt = ysum.reshape(H, cap)
        out_flat[slot_tokens[p][0]] = yt[:, :len(slot_tokens[p][0])].T
        out_flat[slot_tokens[p][1]] = yt[:, B0:B0 + len(slot_tokens[p][1])].T
    return out_flat.reshape(B, S, H)

